# revision 8
# baseline (speedup 1.0000x reference)
"""DSTP-RNN (dual-stage two-phase attention RNN) Trainium2 Bass kernel.

Sharding: pure data-parallel over batch. B=1024 split across 8 NeuronCores,
128 batch rows per core = the 128 SBUF partitions. Weights replicated.

Per-core layout choices:
  - LSTM state kept transposed: hT/cT are [H=128 part, B=128 free]; hT carries
    2*h and every weight that consumes h is pre-scaled by 0.5 host-side, which
    lets the sigmoid come out of the tanh table (sigmoid(z)=0.5+0.5*tanh(z/2))
    without a per-step affine fixup. Only the exp/tanh activation-table set is
    ever loaded (no 2.7us table swaps inside the scan).
  - Attention tensors live as [batch part, (feature, time) free]; broadcast
    adds use stride-0 access patterns; score reduction is a free-axis reduce.
  - Softmax skips max-subtraction (scores are O(1)) and drops Ve*_b / Vd_b
    (softmax is shift-invariant).
"""
import sys
import numpy as np

sys.path.insert(0, "/opt/trn_rl_repo")

import concourse.bass as bass
import concourse.mybir as mybir
from concourse.tile import TileContext

FP = mybir.dt.float32
AX = mybir.AxisListType
OP = mybir.AluOpType
AF = mybir.ActivationFunctionType

B, T, TD, H, NI = 1024, 48, 24, 128, 18
F1, F2, ND = 17, 129, 30          # stage1 feats, stage2 feats, decoder steps
NCORES = 8
BL = B // NCORES                   # 128 batch rows per core

# chunking of the big attention passes (SBUF scratch economy + pipelining)
F2_CH = [(0, 43), (43, 43), (86, 43)]          # stage2 feature chunks
TD_CH = [(0, 16), (16, 16), (32, 16)]          # decoder time chunks
DIN_CH = [(0, 32), (32, 32), (64, 32), (96, 32)]  # decoder h chunks


def _decl(nc, name, shape):
    return nc.dram_tensor(name, list(shape), FP, kind="ExternalInput")


def _split_matmul_waits(nc):
    """Walrus's self-loading Matmult (S3_LW) holds only one sync-wait slot.
    Move every multi-wait matmul's waits onto a PE NoOp inserted just before
    it (same queue, program order preserved)."""
    f = nc.m.functions[0]
    new_blocks = []
    for blk in f.blocks:
        insts = blk.instructions
        out = []
        changed = False
        for ins in insts:
            si = ins.sync_info
            if (si is not None and len(si.on_wait) > 1
                    and type(ins).__name__ != "InstEventSemaphore"
                    and getattr(ins, "engine", None) is not None
                    and str(ins.engine) != "EngineType.Unassigned"):
                waits = list(si.on_wait)
                for k, wt in enumerate(waits[:-1]):
                    ev = mybir.InstEventSemaphore(name=f"{ins.name}-wt{k}")
                    ev.engine = ins.engine
                    ev.sync_info = mybir.SyncInfo(on_wait=[wt], on_update=[])
                    out.append(ev)
                ins.sync_info = mybir.SyncInfo(
                    on_wait=[waits[-1]], on_update=list(si.on_update))
                changed = True
            out.append(ins)
        if changed:
            nb = mybir.BasicBlock(name=blk.name, instructions=out)
            for attr in ("IsExit", "IsLoopEntry", "IsPredicated"):
                val = getattr(blk, attr)
                if val is not None:
                    setattr(nb, attr, val)
            new_blocks.append(nb)
        else:
            new_blocks.append(blk)
    f.blocks = new_blocks


def build_bass():
    nc = bass.Bass()

    # ---------------- DRAM I/O ----------------
    d = {}
    ins = [
        ("xTT", [T, F1 * BL]),        # xTT[t, f*BL+b] = x[b,t,f]
        ("xT", [F1, T * BL]),         # xT[f, t*BL+b] = x[b,t,f]
        ("lab", [BL, T]),
        ("labT", [T, BL]),
        ("we1T_h", [H, T]), ("we1T_c", [H, T]),
        ("ue1T", [T, T]), ("ue1b_rep", [BL, T]), ("ve1_rep", [BL, T]),
        ("wih1T", [F1, 4 * H]), ("whh1T", [H, 4 * H]), ("bias1", [H, 4]),
        ("we2T_h", [H, T]), ("we2T_c", [H, T]),
        ("ue2T", [T, T]), ("ue2b_rep", [BL, T]), ("ve2_rep", [BL, T]),
        ("wih2T_h", [H, 4 * H]), ("wih2T_l", [1, 4 * H]),
        ("whh2T", [H, 4 * H]), ("bias2", [H, 4]),
        ("udT", [H, H]), ("udb_rep", [BL, H]),
        ("wdT_h", [H, H]), ("wdT_c", [H, H]), ("vd_rep", [BL, H]),
        ("wihdT", [H, 4 * H]), ("whhdT", [H, 4 * H]), ("biasd", [H, 4]),
        ("regWT", [H, 1]), ("regb_rep", [BL, 1]),
        ("ident", [128, 128]),
    ]
    for name, shape in ins:
        d[name] = _decl(nc, name, shape)
    out_d = nc.dram_tensor("out", [BL, TD], FP, kind="ExternalOutput")

    with TileContext(nc) as tc:
        with (
            tc.tile_pool(name="pw", bufs=1) as pw,
            tc.tile_pool(name="pst", bufs=1) as pst,
            tc.tile_pool(name="psm", bufs=2) as psm,      # small per-step sbuf
            tc.tile_pool(name="pio", bufs=1) as pio,
            tc.tile_pool(name="pps", bufs=2, space="PSUM") as ppsum,
            tc.tile_pool(name="ptr", bufs=3, space="PSUM") as ptr,
        ):
            # ---- load weights/constants into SBUF ----
            w = {}
            for name, shape in ins:
                if name in ("xTT", "xT"):
                    continue
                tile = pw.tile(list(shape), FP, name="w_" + name, tag="w_" + name)
                nc.gpsimd.dma_start(out=tile[:, :], in_=d[name][:, :])
                w[name] = tile

            # persistent big buffers
            fin_sb = pio.tile([BL, T * H], FP, name="fin_sb", tag="fin_sb")
            out_sb = pio.tile([BL, ND], FP, name="out_sb", tag="out_sb")
            pre2 = pio.tile([BL, F2 * T], FP, name="pre2", tag="pre2")

            # state tiles
            def new_state(pfx):
                hT = pst.tile([H, BL], FP, name=pfx + "h0", tag=pfx + "h", bufs=2)
                cT = pst.tile([H, BL], FP, name=pfx + "c0", tag=pfx + "c", bufs=2)
                nc.vector.memset(hT[:, :], 0.0)
                nc.vector.memset(cT[:, :], 0.0)
                return hT, cT

            ident = w["ident"]

            def lstm_step(pfx, t, xsT, hT, cT, wihT, whhT, bias, xlabT=None,
                          wlabT=None):
                """One transposed LSTM step; returns (hT2=2h, cT2)."""
                gps = ppsum.tile([H, 4 * H], FP, name=f"{pfx}g{t}", tag="gates")
                for g in range(4):
                    gs = slice(g * H, (g + 1) * H)
                    nc.tensor.matmul(gps[:, gs], wihT[:, gs], xsT,
                                     start=True, stop=False)
                    if xlabT is not None:
                        nc.tensor.matmul(gps[:, gs], wlabT[:, gs], xlabT,
                                         start=False, stop=False)
                    nc.tensor.matmul(gps[:, gs], whhT[:, gs], hT,
                                     start=False, stop=True)
                acts = []
                for g, scale in ((0, 0.5), (1, 0.5), (2, 1.0), (3, 0.5)):
                    a = psm.tile([H, BL], FP, name=f"{pfx}a{g}_{t}",
                                 tag=f"act{g}")
                    nc.scalar.activation(a[:, :], gps[:, g * H:(g + 1) * H],
                                         AF.Tanh, bias=bias[:, g:g + 1],
                                         scale=scale)
                    acts.append(a)
                Bv, A, C, D = acts      # i, f, g, o
                u = psm.tile([H, BL], FP, name=f"{pfx}u{t}", tag="lu")
                nc.vector.scalar_tensor_tensor(u[:, :], A[:, :], 1.0, cT[:, :],
                                               op0=OP.add, op1=OP.mult)
                v = psm.tile([H, BL], FP, name=f"{pfx}v{t}", tag="lv")
                nc.vector.scalar_tensor_tensor(v[:, :], Bv[:, :], 1.0, C[:, :],
                                               op0=OP.add, op1=OP.mult)
                ww = psm.tile([H, BL], FP, name=f"{pfx}w{t}", tag="lw")
                nc.vector.tensor_add(ww[:, :], u[:, :], v[:, :])
                cT2 = pst.tile([H, BL], FP, name=f"{pfx}c{t}", tag=pfx + "c",
                               bufs=2)
                nc.vector.tensor_scalar_mul(cT2[:, :], ww[:, :], 0.5)
                Tc = psm.tile([H, BL], FP, name=f"{pfx}T{t}", tag="lTc")
                nc.scalar.activation(Tc[:, :], ww[:, :], AF.Tanh, scale=0.5)
                hT2 = pst.tile([H, BL], FP, name=f"{pfx}h{t}", tag=pfx + "h",
                               bufs=2)
                nc.vector.scalar_tensor_tensor(hT2[:, :], D[:, :], 1.0,
                                               Tc[:, :], op0=OP.add,
                                               op1=OP.mult)
                return hT2, cT2

            def softmax_and_smT(pfx, t, score, F, psm_pool):
                """exp-normalize (no max shift); returns sm [BL,F] sbuf."""
                exps = psm_pool.tile([BL, F], FP, name=f"{pfx}e{t}",
                                     tag="exps")
                sume = psm_pool.tile([BL, 1], FP, name=f"{pfx}se{t}",
                                     tag="sume")
                nc.scalar.activation(exps[:, :], score[:, :], AF.Exp,
                                     accum_out=sume[:, :])
                rs = psm_pool.tile([BL, 1], FP, name=f"{pfx}rs{t}",
                                   tag="rs")
                nc.vector.reciprocal(rs[:, :], sume[:, :])
                sm = psm_pool.tile([BL, F], FP, name=f"{pfx}sm{t}",
                                   tag="sm")
                nc.vector.tensor_scalar_mul(sm[:, :], exps[:, :], rs[:, :])
                return sm

            # ======================= STAGE 1 =======================
            mid_sb = pio.tile([BL, T * F2], FP, name="mid_sb", tag="mid_sb")
            with tc.tile_pool(name="ps1", bufs=1) as ps1:
                xT_sb = ps1.tile([F1, T * BL], FP, name="xT_sb", tag="xT_sb")
                nc.gpsimd.dma_start(out=xT_sb[:, :], in_=d["xT"][:, :])
                pre1 = ps1.tile([BL, F1 * T], FP, name="pre1", tag="pre1")

                # ---- pre1 build: per f, pre1[:,f,:] = xTT_f.T @ ue1T + b ----
                with tc.tile_pool(name="pxtt", bufs=1) as pxtt:
                    xTT_sb = pxtt.tile([T, F1 * BL], FP, name="xTT_sb",
                                       tag="xTT_sb")
                    nc.gpsimd.dma_start(out=xTT_sb[:, :], in_=d["xTT"][:, :])
                    for f in range(F1):
                        pp = ppsum.tile([BL, T], FP, name=f"pp1_{f}",
                                        tag="mm_small")
                        nc.tensor.matmul(pp[:, :],
                                         xTT_sb[:, f * BL:(f + 1) * BL],
                                         w["ue1T"][:, :], start=True, stop=True)
                        nc.vector.scalar_tensor_tensor(
                            pre1[:, f * T:(f + 1) * T], pp[:, :], 1.0,
                            w["ue1b_rep"][:, :], op0=OP.mult, op1=OP.add)

                # label column of mid_sb + stage-1 scan
                nc.vector.tensor_copy(
                    mid_sb[:, :].rearrange("p (t f) -> p t f", f=F2)[:, :, H],
                    w["lab"][:, :])
                hT, cT = new_state("s1")
                for t in range(T):
                    weps = ppsum.tile([BL, T], FP, name=f"we1_{t}",
                                      tag="mm_small")
                    nc.tensor.matmul(weps[:, :], hT[:, :], w["we1T_h"][:, :],
                                     start=True, stop=False)
                    nc.tensor.matmul(weps[:, :], cT[:, :], w["we1T_c"][:, :],
                                     start=False, stop=True)
                    earg = ps1.tile([BL, F1 * T], FP, name=f"ea1_{t}",
                                    tag="big1A", bufs=2)
                    nc.vector.tensor_add(
                        earg[:, :].rearrange("p (f s) -> p f s", s=T),
                        pre1[:, :].rearrange("p (f s) -> p f s", s=T),
                        weps[:, :].unsqueeze(1).broadcast_to([BL, F1, T]))
                    etan = ps1.tile([BL, F1 * T], FP, name=f"et1_{t}",
                                    tag="big1B", bufs=2)
                    nc.scalar.activation(etan[:, :], earg[:, :], AF.Tanh)
                    t2 = ps1.tile([BL, F1 * T], FP, name=f"t21_{t}",
                                  tag="big1A", bufs=2)
                    nc.vector.tensor_mul(
                        t2[:, :].rearrange("p (f s) -> p f s", s=T),
                        etan[:, :].rearrange("p (f s) -> p f s", s=T),
                        w["ve1_rep"][:, :].unsqueeze(1)
                        .broadcast_to([BL, F1, T]))
                    score = psm.tile([BL, F1], FP, name=f"sc1_{t}", tag="sc")
                    nc.vector.reduce_sum(
                        score[:, :],
                        t2[:, :].rearrange("p (f s) -> p f s", s=T), axis=AX.X)
                    sm = softmax_and_smT("s1", t, score, F1, psm)
                    smT = ptr.tile([F1, BL], FP, name=f"smT1_{t}", tag="tr")
                    nc.tensor.transpose(smT[:, :], sm[:, :], ident[:, :])
                    xsT = psm.tile([F1, BL], FP, name=f"xsT1_{t}", tag="xsT")
                    nc.vector.tensor_mul(xsT[:, :],
                                         xT_sb[:, t * BL:(t + 1) * BL],
                                         smT[:, :])
                    hT, cT = lstm_step("s1", t, xsT, hT, cT, w["wih1T"],
                                       w["whh1T"], w["bias1"])
                    # store mid (true h = 0.5*hT)
                    hbt = ptr.tile([BL, H], FP, name=f"hbt_{t}", tag="tr")
                    nc.tensor.transpose(hbt[:, :], hT[:, :], ident[:, :])
                    nc.vector.tensor_scalar_mul(
                        mid_sb[:, t * F2:t * F2 + H], hbt[:, :], 0.5)

                # ---- pre2 build: per f, transpose mid column then matmul ----
                mid3 = mid_sb[:, :].rearrange("p (t f) -> p t f", f=F2)
                for f in range(F2):
                    mtp = ptr.tile([T, BL], FP, name=f"mtp_{f}", tag="tr")
                    nc.tensor.transpose(mtp[:, :], mid3[:, :, f], ident[:, :])
                    msc = psm.tile([T, BL], FP, name=f"msc_{f}", tag="prescr")
                    nc.vector.tensor_copy(msc[:, :], mtp[:, :])
                    pp = ppsum.tile([BL, T], FP, name=f"pp2_{f}",
                                    tag="mm_small")
                    nc.tensor.matmul(pp[:, :], msc[:, :], w["ue2T"][:, :],
                                     start=True, stop=True)
                    nc.vector.scalar_tensor_tensor(
                        pre2[:, f * T:(f + 1) * T], pp[:, :], 1.0,
                        w["ue2b_rep"][:, :], op0=OP.mult, op1=OP.add)

            # ======================= STAGE 2 =======================
            with tc.tile_pool(name="ps2", bufs=1) as ps2:
                hT, cT = new_state("s2")
                for t in range(T):
                    weps = ppsum.tile([BL, T], FP, name=f"we2_{t}",
                                      tag="mm_small")
                    nc.tensor.matmul(weps[:, :], hT[:, :], w["we2T_h"][:, :],
                                     start=True, stop=False)
                    nc.tensor.matmul(weps[:, :], cT[:, :], w["we2T_c"][:, :],
                                     start=False, stop=True)
                    score = psm.tile([BL, F2], FP, name=f"sc2_{t}", tag="sc")
                    for ci, (f0, fn) in enumerate(F2_CH):
                        earg = ps2.tile([BL, fn * T], FP, name=f"ea2_{t}_{ci}",
                                        tag="big2A", bufs=2)
                        nc.vector.tensor_add(
                            earg[:, :].rearrange("p (f s) -> p f s", s=T),
                            pre2[:, f0 * T:(f0 + fn) * T].rearrange(
                                "p (f s) -> p f s", s=T),
                            weps[:, :].unsqueeze(1).broadcast_to([BL, fn, T]))
                        etan = ps2.tile([BL, fn * T], FP, name=f"et2_{t}_{ci}",
                                        tag="big2B", bufs=2)
                        nc.scalar.activation(etan[:, :], earg[:, :], AF.Tanh)
                        t2 = ps2.tile([BL, fn * T], FP, name=f"t22_{t}_{ci}",
                                      tag="big2A", bufs=2)
                        nc.vector.tensor_mul(
                            t2[:, :].rearrange("p (f s) -> p f s", s=T),
                            etan[:, :].rearrange("p (f s) -> p f s", s=T),
                            w["ve2_rep"][:, :].unsqueeze(1)
                            .broadcast_to([BL, fn, T]))
                        nc.vector.reduce_sum(
                            score[:, f0:f0 + fn],
                            t2[:, :].rearrange("p (f s) -> p f s", s=T),
                            axis=AX.X)
                    sm = softmax_and_smT("s2", t, score, F2, psm)
                    smT = ptr.tile([H, BL], FP, name=f"smT2_{t}", tag="tr")
                    nc.tensor.transpose(smT[:, :], sm[:, 0:H], ident[:, :])
                    smT_sb = psm.tile([H, BL], FP, name=f"smTs_{t}",
                                      tag="smT_sb")
                    nc.scalar.copy(smT_sb[:, :], smT[:, :])
                    mTp = ptr.tile([H, BL], FP, name=f"mTp_{t}", tag="tr")
                    nc.tensor.transpose(
                        mTp[:, :], mid_sb[:, t * F2:t * F2 + H], ident[:, :])
                    xsT = psm.tile([H, BL], FP, name=f"xsT2_{t}", tag="xsT")
                    nc.vector.tensor_mul(xsT[:, :], mTp[:, :], smT_sb[:, :])
                    xlab = psm.tile([BL, 1], FP, name=f"xl_{t}", tag="s2xl")
                    nc.vector.tensor_mul(xlab[:, :], w["lab"][:, t:t + 1],
                                         sm[:, H:H + 1])
                    xlT_ps = ptr.tile([1, BL], FP, name=f"xlT_{t}", tag="tr")
                    nc.tensor.transpose(xlT_ps[:, :], xlab[:, :], ident[:, :])
                    xlabT = psm.tile([1, BL], FP, name=f"xls_{t}", tag="s2xls")
                    nc.vector.tensor_copy(xlabT[:, :], xlT_ps[:, :])
                    hT, cT = lstm_step("s2", t, xsT, hT, cT, w["wih2T_h"],
                                       w["whh2T"], w["bias2"], xlabT=xlabT,
                                       wlabT=w["wih2T_l"])
                    hbt = ptr.tile([BL, H], FP, name=f"hbt2_{t}", tag="tr")
                    nc.tensor.transpose(hbt[:, :], hT[:, :], ident[:, :])
                    nc.vector.tensor_scalar_mul(fin_sb[:, t * H:(t + 1) * H],
                                                hbt[:, :], 0.5)

            # ======================= DECODER =======================
            with tc.tile_pool(name="pdec", bufs=1) as pdec:
                ud = pdec.tile([BL, T * H], FP, name="ud", tag="ud")
                for t in range(T):
                    ftp = ptr.tile([H, BL], FP, name=f"ftp_{t}", tag="tr")
                    nc.tensor.transpose(ftp[:, :], fin_sb[:, t * H:(t + 1) * H],
                                        ident[:, :])
                    fts = psm.tile([H, BL], FP, name=f"fts_{t}", tag="smT_sb")
                    nc.vector.tensor_copy(fts[:, :], ftp[:, :])
                    up = ppsum.tile([BL, H], FP, name=f"udp_{t}",
                                    tag="mm_small")
                    nc.tensor.matmul(up[:, :], fts[:, :],
                                     w["udT"][:, :], start=True, stop=True)
                    nc.vector.scalar_tensor_tensor(
                        ud[:, t * H:(t + 1) * H], up[:, :], 1.0,
                        w["udb_rep"][:, :], op0=OP.mult, op1=OP.add)

                hT, cT = new_state("sd")
                for t in range(ND):
                    wdps = ppsum.tile([BL, H], FP, name=f"wd_{t}",
                                      tag="mm_small")
                    nc.tensor.matmul(wdps[:, :], hT[:, :], w["wdT_h"][:, :],
                                     start=True, stop=False)
                    nc.tensor.matmul(wdps[:, :], cT[:, :], w["wdT_c"][:, :],
                                     start=False, stop=True)
                    score = psm.tile([BL, T], FP, name=f"scd_{t}", tag="sc")
                    for ci, (t0, tn) in enumerate(TD_CH):
                        earg = pdec.tile([BL, tn * H], FP,
                                         name=f"ead_{t}_{ci}", tag="bigdA",
                                         bufs=2)
                        nc.vector.tensor_add(
                            earg[:, :].rearrange("p (s h) -> p s h", h=H),
                            ud[:, t0 * H:(t0 + tn) * H].rearrange(
                                "p (s h) -> p s h", h=H),
                            wdps[:, :].unsqueeze(1).broadcast_to([BL, tn, H]))
                        etan = pdec.tile([BL, tn * H], FP,
                                         name=f"etd_{t}_{ci}", tag="bigdB",
                                         bufs=2)
                        nc.scalar.activation(etan[:, :], earg[:, :], AF.Tanh)
                        t2 = pdec.tile([BL, tn * H], FP, name=f"t2d_{t}_{ci}",
                                       tag="bigdA", bufs=2)
                        nc.vector.tensor_mul(
                            t2[:, :].rearrange("p (s h) -> p s h", h=H),
                            etan[:, :].rearrange("p (s h) -> p s h", h=H),
                            w["vd_rep"][:, :].unsqueeze(1)
                            .broadcast_to([BL, tn, H]))
                        nc.vector.reduce_sum(
                            score[:, t0:t0 + tn],
                            t2[:, :].rearrange("p (s h) -> p s h", h=H),
                            axis=AX.X)
                    sm = softmax_and_smT("sd", t, score, T, psm)
                    din = psm.tile([BL, H], FP, name=f"din_{t}", tag="sddin")
                    for ci, (h0, hn) in enumerate(DIN_CH):
                        dmul = pdec.tile([BL, T * hn], FP,
                                         name=f"dm_{t}_{ci}", tag="bigdA",
                                         bufs=2)
                        nc.vector.tensor_mul(
                            dmul[:, :].rearrange("p (s h) -> p s h", h=hn),
                            fin_sb[:, :].rearrange(
                                "p (s h) -> p s h", h=H)[:, :, h0:h0 + hn],
                            sm[:, :].unsqueeze(2).broadcast_to([BL, T, hn]))
                        nc.vector.reduce_sum(
                            din[:, h0:h0 + hn],
                            dmul[:, :].rearrange("p (s h) -> p h s", h=hn),
                            axis=AX.X)
                    dinT_ps = ptr.tile([H, BL], FP, name=f"dTp_{t}", tag="tr")
                    nc.tensor.transpose(dinT_ps[:, :], din[:, :], ident[:, :])
                    dinT = psm.tile([H, BL], FP, name=f"dT_{t}", tag="sddinT")
                    nc.vector.tensor_copy(dinT[:, :], dinT_ps[:, :])
                    hT, cT = lstm_step("sd", t, dinT, hT, cT, w["wihdT"],
                                       w["whhdT"], w["biasd"])
                    op = ppsum.tile([BL, 1], FP, name=f"op_{t}",
                                    tag="mm_small")
                    nc.tensor.matmul(op[:, :], hT[:, :], w["regWT"][:, :],
                                     start=True, stop=True)
                    nc.vector.tensor_copy(out_sb[:, t:t + 1], op[:, :])

                outf = pdec.tile([BL, TD], FP, name="outf", tag="outf")
                nc.vector.tensor_scalar_add(outf[:, :], out_sb[:, 6:ND],
                                            w["regb_rep"][:, :])
                nc.gpsimd.dma_start(out=out_d[:, :], in_=outf[:, :])

    _split_matmul_waits(nc)
    return nc


# ---------------- host-side prep ----------------
def _prep_weights(i):
    f32 = lambda a: np.ascontiguousarray(a, np.float32)
    w = {}
    w["we1T_h"] = f32(0.5 * i["We1_W"][:, :H].T)
    w["we1T_c"] = f32(i["We1_W"][:, H:].T)
    w["ue1T"] = f32(i["Ue1_W"].T)
    w["ue1b_rep"] = f32(np.tile(i["Ue1_b"][None, :], (BL, 1)))
    w["ve1_rep"] = f32(np.tile(i["Ve1_W"][0][None, :], (BL, 1)))
    w["wih1T"] = f32(i["e1_Wih"].T)
    w["whh1T"] = f32(0.5 * i["e1_Whh"].T)
    b1 = i["e1_bih"] + i["e1_bhh"]
    w["bias1"] = f32(np.stack([0.5 * b1[0:H], 0.5 * b1[H:2 * H],
                               b1[2 * H:3 * H], 0.5 * b1[3 * H:]], 1))
    w["we2T_h"] = f32(0.5 * i["We2_W"][:, :H].T)
    w["we2T_c"] = f32(i["We2_W"][:, H:].T)
    w["ue2T"] = f32(i["Ue2_W"].T)
    w["ue2b_rep"] = f32(np.tile(i["Ue2_b"][None, :], (BL, 1)))
    w["ve2_rep"] = f32(np.tile(i["Ve2_W"][0][None, :], (BL, 1)))
    w["wih2T_h"] = f32(i["e2_Wih"][:, :H].T)
    w["wih2T_l"] = f32(i["e2_Wih"][:, H:].T)
    w["whh2T"] = f32(0.5 * i["e2_Whh"].T)
    b2 = i["e2_bih"] + i["e2_bhh"]
    w["bias2"] = f32(np.stack([0.5 * b2[0:H], 0.5 * b2[H:2 * H],
                               b2[2 * H:3 * H], 0.5 * b2[3 * H:]], 1))
    w["udT"] = f32(i["Ud_W"].T)
    w["udb_rep"] = f32(np.tile(i["Ud_b"][None, :], (BL, 1)))
    w["wdT_h"] = f32(0.5 * i["Wd_W"][:, :H].T)
    w["wdT_c"] = f32(i["Wd_W"][:, H:].T)
    w["vd_rep"] = f32(np.tile(i["Vd_W"][0][None, :], (BL, 1)))
    w["wihdT"] = f32(i["d_Wih"].T)
    w["whhdT"] = f32(0.5 * i["d_Whh"].T)
    bd = i["d_bih"] + i["d_bhh"]
    w["biasd"] = f32(np.stack([0.5 * bd[0:H], 0.5 * bd[H:2 * H],
                               bd[2 * H:3 * H], 0.5 * bd[3 * H:]], 1))
    w["regWT"] = f32(0.5 * i["reg_W"].T)
    w["regb_rep"] = f32(np.tile(i["reg_b"][None, :], (BL, 1)))
    w["ident"] = np.eye(128, dtype=np.float32)
    return w


def prep_in_maps(inputs):
    i = {k: np.asarray(v) for k, v in inputs.items()}
    wshared = _prep_weights(i)
    maps = []
    for c in range(NCORES):
        sl = slice(c * BL, (c + 1) * BL)
        x = np.ascontiguousarray(i["input_p_q"][sl][:, :, 1:], np.float32)
        lab = np.ascontiguousarray(i["label_p"][sl], np.float32)
        m = dict(wshared)
        m["xTT"] = np.ascontiguousarray(
            x.transpose(1, 2, 0).reshape(T, F1 * BL))
        m["xT"] = np.ascontiguousarray(
            x.transpose(2, 1, 0).reshape(F1, T * BL))
        m["lab"] = lab
        m["labT"] = np.ascontiguousarray(lab.T)
        maps.append(m)
    return maps


_CACHE = {}


def kernel(**inputs):
    from concourse.bass_utils import run_bass_kernel_spmd
    if "nc" not in _CACHE:
        _CACHE["nc"] = build_bass()
    nc = _CACHE["nc"]
    in_maps = prep_in_maps(inputs)
    res = run_bass_kernel_spmd(nc, in_maps, list(range(NCORES)))
    out = np.concatenate([res.results[c]["out"] for c in range(NCORES)], 0)
    return out.astype(np.float32)


if __name__ == "__main__":
    nc = build_bass()
    print("built ok")


# revision 21
# speedup vs baseline: 41.4835x; 41.4835x over previous
"""DSTP-RNN (dual-stage two-phase attention RNN) Trainium2 Bass kernel.

Sharding: pure data-parallel over batch. B=1024 split across 8 NeuronCores,
128 batch rows per core = the 128 SBUF partitions. Weights replicated.

Per-core design:
  - LSTM state transposed: hT/cT are [H=128 part, B=128 free] carrying 2*h and
    2*c; weights consuming them are pre-scaled 0.5 host-side. This makes
    sigmoid available from the tanh table (sigmoid(z)=0.5+0.5*tanh(z/2))
    with zero per-step fixups, and the cell update is 4 fused DVE ops.
    Only the exp/tanh activation-table set is ever loaded. h is carried in
    bf16 (feeds only matmuls / attention); c stays fp32 (it accumulates).
  - Attention tensors are [batch part, (feature, time) free] bf16; broadcast
    adds/muls run in the DVE 2x perf mode; score reductions use bf16
    halving trees; softmax skips max-subtraction (scores are O(1)) and drops
    Ve*_b / Vd_b (softmax shift-invariance).
  - All big matmul operands are bf16 (fp32 matmul is 4 cyc/row; bf16 is 1);
    the small c-side projection tiles stay fp32 to skip down-converts.
"""
import sys
import numpy as np

sys.path.insert(0, "/opt/trn_rl_repo")

import concourse.bass as bass
import concourse.mybir as mybir
from concourse.tile import TileContext

FP = mybir.dt.float32
BF = mybir.dt.bfloat16
AX = mybir.AxisListType
OP = mybir.AluOpType
AF = mybir.ActivationFunctionType

B, T, TD, H, NI = 1024, 48, 24, 128, 18
F1, F2, ND = 17, 129, 30
NCORES = 8
BL = B // NCORES

F2_CH = [(0, 43), (43, 43), (86, 43)]
TD_CH = [(0, 16), (16, 16), (32, 16)]
DIN_CH = [(0, 64), (64, 64)]


def _split_matmul_waits(nc):
    """Walrus allows one sync-wait slot per instruction. Hoist extra waits
    onto same-engine EventSemaphore instructions inserted just before."""
    f = nc.m.functions[0]
    new_blocks = []
    for blk in f.blocks:
        insts = blk.instructions
        out = []
        changed = False
        for ins in insts:
            si = ins.sync_info
            if (si is not None and len(si.on_wait) > 1
                    and type(ins).__name__ != "InstEventSemaphore"
                    and getattr(ins, "engine", None) is not None
                    and str(ins.engine) != "EngineType.Unassigned"):
                waits = list(si.on_wait)
                for k, wt in enumerate(waits[:-1]):
                    ev = mybir.InstEventSemaphore(name=f"{ins.name}-wt{k}")
                    ev.engine = ins.engine
                    ev.sync_info = mybir.SyncInfo(on_wait=[wt], on_update=[])
                    out.append(ev)
                ins.sync_info = mybir.SyncInfo(
                    on_wait=[waits[-1]], on_update=list(si.on_update))
                changed = True
            out.append(ins)
        if changed:
            nb = mybir.BasicBlock(name=blk.name, instructions=out)
            for attr in ("IsExit", "IsLoopEntry", "IsPredicated"):
                val = getattr(blk, attr)
                if val is not None:
                    setattr(nb, attr, val)
            new_blocks.append(nb)
        else:
            new_blocks.append(blk)
    f.blocks = new_blocks


INS = [
    ("xTT", [T, F1 * BL], BF),      # xTT[t, f*BL+b] = x[b,t,f]
    ("xT", [F1, T * BL], BF),       # xT[f, t*BL+b] = x[b,t,f]
    ("lab", [BL, T], FP),
    ("we1T_h", [H, T], BF), ("we1T_c", [H, T], FP),
    ("ue1T", [T, T], BF), ("ue1b_rep", [BL, T], FP), ("ve1_rep", [BL, T], BF),
    ("wih1T", [F1, 4 * H], BF), ("whh1T", [H, 4 * H], BF), ("bias1", [2, 4 * H], BF),
    ("we2T_h", [H, T], BF), ("we2T_c", [H, T], FP),
    ("ue2T", [T, T], BF), ("ue2b_rep", [BL, T], FP), ("ve2_rep", [BL, T], BF),
    ("wih2T_h", [H, 4 * H], BF), ("wih2T_l", [1, 4 * H], BF),
    ("whh2T", [H, 4 * H], BF), ("bias2", [2, 4 * H], BF),
    ("udT", [H, H], BF), ("udb_rep", [BL, H], FP),
    ("wdT_h", [H, H], BF), ("wdT_c", [H, H], FP), ("vd_rep", [BL, H], BF),
    ("wihdT", [H, 4 * H], BF), ("whhdT", [H, 4 * H], BF), ("biasd", [2, 4 * H], BF),
    ("regWT", [H, 1], FP), ("regb_rep", [BL, 1], FP),
    ("identf", [128, 128], FP), ("identb", [128, 128], BF),
    ("ones_row", [2, BL], BF),
]


def build_bass(reps=1, split_waits=True):
    nc = bass.Bass()
    d = {name: nc.dram_tensor(name, list(shape), dt, kind="ExternalInput")
         for name, shape, dt in INS}
    out_d = nc.dram_tensor("out", [BL, TD], FP, kind="ExternalOutput")

    with TileContext(nc) as tc:
        with (
            tc.tile_pool(name="pw", bufs=1) as pw,
            tc.tile_pool(name="pst", bufs=1) as pst,
            tc.tile_pool(name="psm", bufs=3) as psm,
            tc.tile_pool(name="pio", bufs=1) as pio,
            tc.tile_pool(name="pps", bufs=2, space="PSUM") as ppsum,
            tc.tile_pool(name="ptr", bufs=4, space="PSUM") as ptr,
        ):
            w = {}
            for name, shape, dt in INS:
                if name in ("xTT", "xT"):
                    continue
                tile = pw.tile(list(shape), dt, name="w_" + name,
                               tag="w_" + name)
                nc.gpsimd.dma_start(out=tile[:, :], in_=d[name][:, :])
                w[name] = tile
            idf, idb = w["identf"], w["identb"]

            def new_state(pfx, hdt):
                hT = pst.tile([H, BL], hdt, name=pfx + "h0", tag=pfx + "h",
                              bufs=2)
                cT = pst.tile([H, BL], FP, name=pfx + "c0", tag=pfx + "c",
                              bufs=2)
                nc.vector.memset(hT[:, :], 0.0)
                nc.vector.memset(cT[:, :], 0.0)
                return hT, cT

            def lstm_begin(pfx, t, hTb, whhT):
                """Record the h-side operands; matmuls issue in finish."""
                return (hTb, whhT)

            def lstm_finish(pfx, t, gps, xsT, cT, wihT, biasrow, hdt,
                            xlabT=None, wlabT=None):
                """Gate order [i,f,o,g]; bias enters PSUM via K=1 matmuls so
                i/f/o share one merged tanh(0.5*..) activation."""
                hTb, whhT = gps
                gps = ppsum.tile([H, 4 * H], FP, name=f"{pfx}g{t}",
                                 tag="gates")
                for g in range(4):
                    gs = slice(g * H, (g + 1) * H)
                    nc.tensor.matmul(gps[:, gs], whhT[:, gs], hTb[:, :],
                                     start=True, stop=False)
                    nc.tensor.matmul(gps[:, gs], biasrow[:, gs],
                                     w["ones_row"][:, :], start=False,
                                     stop=False)
                    if xlabT is not None:
                        nc.tensor.matmul(gps[:, gs], wlabT[:, gs], xlabT,
                                         start=False, stop=False)
                    nc.tensor.matmul(gps[:, gs], wihT[:, gs], xsT,
                                     start=False, stop=True)
                Ab = psm.tile([H, 3 * BL], FP, name=f"{pfx}ab{t}", tag="actA")
                nc.scalar.activation(Ab[:, :], gps[:, 0:3 * H], AF.Tanh,
                                     scale=0.5)
                C = psm.tile([H, BL], FP, name=f"{pfx}cg{t}", tag="actC")
                nc.scalar.activation(C[:, :], gps[:, 3 * H:4 * H], AF.Tanh)
                Bv, A, D = (Ab[:, 0:BL], Ab[:, BL:2 * BL], Ab[:, 2 * BL:])
                u = psm.tile([H, BL], FP, name=f"{pfx}u{t}", tag="lu")
                nc.vector.scalar_tensor_tensor(u[:, :], A, 1.0,
                                               cT[:, :], op0=OP.add,
                                               op1=OP.mult)
                v = psm.tile([H, BL], FP, name=f"{pfx}v{t}", tag="lv")
                nc.vector.scalar_tensor_tensor(v[:, :], Bv, 1.0,
                                               C[:, :], op0=OP.add,
                                               op1=OP.mult)
                cT2 = pst.tile([H, BL], FP, name=f"{pfx}c{t}", tag=pfx + "c",
                               bufs=2)
                nc.vector.scalar_tensor_tensor(cT2[:, :], u[:, :], 0.5,
                                               v[:, :], op0=OP.mult,
                                               op1=OP.add)
                Tc = psm.tile([H, BL], FP, name=f"{pfx}T{t}", tag="lTc")
                nc.scalar.activation(Tc[:, :], cT2[:, :], AF.Tanh, scale=0.5)
                hT2 = pst.tile([H, BL], hdt, name=f"{pfx}h{t}",
                               tag=pfx + "h", bufs=2)
                nc.vector.scalar_tensor_tensor(hT2[:, :], D, 1.0,
                                               Tc[:, :], op0=OP.add,
                                               op1=OP.mult)
                return hT2, cT2

            def softmax(pfx, t, score, F):
                """exp-normalize (no max shift); returns bf16 sm [BL,F]."""
                exps = psm.tile([BL, F], FP, name=f"{pfx}e{t}", tag="exps")
                sume = psm.tile([BL, 1], FP, name=f"{pfx}se{t}", tag="sume")
                nc.scalar.activation(exps[:, :], score[:, :], AF.Exp,
                                     accum_out=sume[:, :])
                rs = psm.tile([BL, 1], FP, name=f"{pfx}rs{t}", tag="rs")
                nc.vector.reciprocal(rs[:, :], sume[:, :])
                sm = psm.tile([BL, F], BF, name=f"{pfx}sm{t}", tag="sm")
                nc.vector.tensor_scalar_mul(sm[:, :], exps[:, :], rs[:, :])
                return sm

            def tree_s(pool, pfx, t, ci, t2, fn, score_sl):
                """sum over innermost S=48 of bf16 [BL, fn, 48]."""
                cur, size = t2, T
                for lvl, ns in enumerate((24, 12, 6)):
                    nxt = pool.tile([BL, fn * ns], BF,
                                    name=f"{pfx}r{t}_{ci}_{lvl}",
                                    tag=f"{pfx}tr{lvl}", bufs=2)
                    cv = cur[:, :].rearrange("p (f s) -> p f s", s=size)
                    nc.vector.tensor_add(
                        nxt[:, :].rearrange("p (f s) -> p f s", s=ns),
                        cv[:, :, 0:ns], cv[:, :, ns:2 * ns])
                    cur, size = nxt, ns
                nc.vector.reduce_sum(
                    score_sl, cur[:, :].rearrange("p (f s) -> p f s", s=6),
                    axis=AX.X)

            def tree_h(pool, pfx, t, ci, t2, tn, score_sl):
                """sum over innermost H=128 of bf16 [BL, tn, 128]."""
                cur, size = t2, H
                for lvl, ns in enumerate((64, 32, 16, 8)):
                    nxt = pool.tile([BL, tn * ns], BF,
                                    name=f"{pfx}q{t}_{ci}_{lvl}",
                                    tag=f"{pfx}th{lvl}", bufs=2)
                    cv = cur[:, :].rearrange("p (s h) -> p s h", h=size)
                    nc.vector.tensor_add(
                        nxt[:, :].rearrange("p (s h) -> p s h", h=ns),
                        cv[:, :, 0:ns], cv[:, :, ns:2 * ns])
                    cur, size = nxt, ns
                nc.vector.reduce_sum(
                    score_sl, cur[:, :].rearrange("p (s h) -> p s h", h=8),
                    axis=AX.X)

            def tree_t(pool, pfx, t, ci, dm, hn, din_sl):
                """sum over outer T=48 of bf16 [BL, 48, hn] (contig blocks)."""
                cur = dm
                for lvl, ns in enumerate((24, 12, 6)):
                    nxt = pool.tile([BL, ns * hn], BF,
                                    name=f"{pfx}d{t}_{ci}_{lvl}",
                                    tag=f"{pfx}tt{lvl}", bufs=2)
                    nc.vector.tensor_add(nxt[:, :], cur[:, 0:ns * hn],
                                         cur[:, ns * hn:2 * ns * hn])
                    cur = nxt
                nc.vector.reduce_sum(
                    din_sl,
                    cur[:, :].rearrange("p (s h) -> p h s", h=hn), axis=AX.X)

            for _rep in range(reps):
                # =================== STAGE 1 ===================
                mid_sb = pio.tile([BL, T * F2], BF, name="mid_sb",
                                  tag="mid_sb")
                fin_sb = pio.tile([BL, T * H], BF, name="fin_sb",
                                  tag="fin_sb")
                out_sb = pio.tile([BL, ND], FP, name="out_sb", tag="out_sb")
                pre2 = pio.tile([BL, F2 * T], BF, name="pre2", tag="pre2")
                with tc.tile_pool(name="ps1", bufs=1) as ps1:
                    xT_sb = ps1.tile([F1, T * BL], BF, name="xT_sb",
                                     tag="xT_sb")
                    nc.gpsimd.dma_start(out=xT_sb[:, :], in_=d["xT"][:, :])
                    pre1 = ps1.tile([BL, F1 * T], BF, name="pre1",
                                    tag="pre1")

                    with tc.tile_pool(name="pxtt", bufs=1) as pxtt:
                        xTT_sb = pxtt.tile([T, F1 * BL], BF, name="xTT_sb",
                                           tag="xTT_sb")
                        nc.gpsimd.dma_start(out=xTT_sb[:, :],
                                            in_=d["xTT"][:, :])
                        for f in range(F1):
                            pp = ppsum.tile([BL, T], FP, name=f"pp1_{f}",
                                            tag="mm_small")
                            nc.tensor.matmul(pp[:, :],
                                             xTT_sb[:, f * BL:(f + 1) * BL],
                                             w["ue1T"][:, :], start=True,
                                             stop=True)
                            nc.vector.scalar_tensor_tensor(
                                pre1[:, f * T:(f + 1) * T], pp[:, :], 1.0,
                                w["ue1b_rep"][:, :], op0=OP.mult, op1=OP.add)

                    nc.vector.tensor_copy(
                        mid_sb[:, :].rearrange("p (t f) -> p t f",
                                               f=F2)[:, :, H],
                        w["lab"][:, :])
                    hT, cT = new_state("s1", BF)
                    for t in range(T):
                        weps = ppsum.tile([BL, T], FP, name=f"we1_{t}",
                                          tag="mm_small")
                        nc.tensor.matmul(weps[:, :], hT[:, :],
                                         w["we1T_h"][:, :], start=True,
                                         stop=False)
                        nc.tensor.matmul(weps[:, :], cT[:, :],
                                         w["we1T_c"][:, :], start=False,
                                         stop=True)
                        gps = lstm_begin("s1", t, hT, w["whh1T"])
                        earg = ps1.tile([BL, F1 * T], BF, name=f"ea1_{t}",
                                        tag="big1A", bufs=2)
                        nc.vector.tensor_add(
                            earg[:, :].rearrange("p (f s) -> p f s", s=T),
                            pre1[:, :].rearrange("p (f s) -> p f s", s=T),
                            weps[:, :].unsqueeze(1).broadcast_to(
                                [BL, F1, T]))
                        etan = ps1.tile([BL, F1 * T], BF, name=f"et1_{t}",
                                        tag="big1B", bufs=2)
                        nc.scalar.activation(etan[:, :], earg[:, :], AF.Tanh)
                        t2 = ps1.tile([BL, F1 * T], BF, name=f"t21_{t}",
                                      tag="big1A", bufs=2)
                        nc.vector.tensor_mul(
                            t2[:, :].rearrange("p (f s) -> p f s", s=T),
                            etan[:, :].rearrange("p (f s) -> p f s", s=T),
                            w["ve1_rep"][:, :].unsqueeze(1).broadcast_to(
                                [BL, F1, T]))
                        score = psm.tile([BL, F1], FP, name=f"sc1_{t}",
                                         tag="sc")
                        nc.vector.reduce_sum(
                            score[:, :],
                            t2[:, :].rearrange("p (f s) -> p f s", s=T),
                            axis=AX.X)
                        sm = softmax("s1", t, score, F1)
                        smT = ptr.tile([F1, BL], BF, name=f"smT1_{t}",
                                       tag="tr")
                        nc.tensor.transpose(smT[:, :], sm[:, :], idb[:, :])
                        xsT = psm.tile([F1, BL], BF, name=f"xsT1_{t}",
                                       tag="xsT")
                        nc.vector.tensor_mul(xsT[:, :],
                                             xT_sb[:, t * BL:(t + 1) * BL],
                                             smT[:, :])
                        hT, cT = lstm_finish("s1", t, gps, xsT, cT,
                                             w["wih1T"], w["bias1"], BF)
                        hbt = ptr.tile([BL, H], BF, name=f"hbt_{t}",
                                       tag="tr")
                        nc.tensor.transpose(hbt[:, :], hT[:, :], idb[:, :])
                        nc.scalar.mul(mid_sb[:, t * F2:t * F2 + H],
                                      hbt[:, :], 0.5)

                    # ---- pre2 build ----
                    mid3 = mid_sb[:, :].rearrange("p (t f) -> p t f", f=F2)
                    for f in range(F2):
                        mtp = ptr.tile([T, BL], BF, name=f"mtp_{f}",
                                       tag="tr")
                        nc.tensor.transpose(mtp[:, :], mid3[:, :, f],
                                            idb[:, :])
                        msc = psm.tile([T, BL], BF, name=f"msc_{f}",
                                       tag="prescr")
                        nc.scalar.copy(msc[:, :], mtp[:, :])
                        pp = ppsum.tile([BL, T], FP, name=f"pp2_{f}",
                                        tag="mm_small")
                        nc.tensor.matmul(pp[:, :], msc[:, :],
                                         w["ue2T"][:, :], start=True,
                                         stop=True)
                        nc.vector.scalar_tensor_tensor(
                            pre2[:, f * T:(f + 1) * T], pp[:, :], 1.0,
                            w["ue2b_rep"][:, :], op0=OP.mult, op1=OP.add)

                # =================== STAGE 2 ===================
                with tc.tile_pool(name="ps2", bufs=1) as ps2:
                    # midT precompute: mid features transposed, off the
                    # per-step critical path
                    midT = ps2.tile([H, T * BL], BF, name="midT", tag="midT")
                    for t in range(T):
                        mq = ptr.tile([H, BL], BF, name=f"mq_{t}", tag="tr")
                        nc.tensor.transpose(
                            mq[:, :], mid_sb[:, t * F2:t * F2 + H],
                            idb[:, :])
                        nc.scalar.copy(midT[:, t * BL:(t + 1) * BL],
                                       mq[:, :])
                    hT, cT = new_state("s2", BF)
                    for t in range(T):
                        weps = ppsum.tile([BL, T], FP, name=f"we2_{t}",
                                          tag="mm_small")
                        nc.tensor.matmul(weps[:, :], hT[:, :],
                                         w["we2T_h"][:, :], start=True,
                                         stop=False)
                        nc.tensor.matmul(weps[:, :], cT[:, :],
                                         w["we2T_c"][:, :], start=False,
                                         stop=True)
                        gps = lstm_begin("s2", t, hT, w["whh2T"])
                        we_sb = psm.tile([BL, T], BF, name=f"wes2_{t}",
                                         tag="we_sb")
                        nc.scalar.copy(we_sb[:, :], weps[:, :])
                        score = psm.tile([BL, F2], FP, name=f"sc2_{t}",
                                         tag="sc")
                        eargs, etans = [], []
                        for ci, (f0, fn) in enumerate(F2_CH):
                            earg = ps2.tile([BL, fn * T], BF,
                                            name=f"ea2_{t}_{ci}",
                                            tag=f"big2A{ci}", bufs=2)
                            nc.vector.tensor_add(
                                earg[:, :].rearrange("p (f s) -> p f s",
                                                     s=T),
                                pre2[:, f0 * T:(f0 + fn) * T].rearrange(
                                    "p (f s) -> p f s", s=T),
                                we_sb[:, :].unsqueeze(1).broadcast_to(
                                    [BL, fn, T]))
                            eargs.append(earg)
                        for ci, (f0, fn) in enumerate(F2_CH):
                            etan = ps2.tile([BL, fn * T], BF,
                                            name=f"et2_{t}_{ci}",
                                            tag=f"big2B{ci}", bufs=2)
                            nc.scalar.activation(etan[:, :],
                                                 eargs[ci][:, :], AF.Tanh)
                            etans.append(etan)
                        for ci, (f0, fn) in enumerate(F2_CH):
                            t2 = ps2.tile([BL, fn * T], BF,
                                          name=f"t22_{t}_{ci}",
                                          tag=f"big2A{ci}", bufs=2)
                            nc.vector.tensor_mul(
                                t2[:, :].rearrange("p (f s) -> p f s", s=T),
                                etans[ci][:, :].rearrange(
                                    "p (f s) -> p f s", s=T),
                                w["ve2_rep"][:, :].unsqueeze(1).broadcast_to(
                                    [BL, fn, T]))
                            tree_s(ps2, "s2", t, ci, t2, fn,
                                   score[:, f0:f0 + fn])
                        sm = softmax("s2", t, score, F2)
                        smT = ptr.tile([H, BL], BF, name=f"smT2_{t}",
                                       tag="tr")
                        nc.tensor.transpose(smT[:, :], sm[:, 0:H],
                                            idb[:, :])
                        xsT = psm.tile([H, BL], BF, name=f"xsT2_{t}",
                                       tag="xsT")
                        nc.vector.tensor_mul(xsT[:, :],
                                             midT[:, t * BL:(t + 1) * BL],
                                             smT[:, :])
                        xlab = psm.tile([BL, 1], FP, name=f"xl_{t}",
                                        tag="s2xl")
                        nc.vector.tensor_mul(xlab[:, :],
                                             w["lab"][:, t:t + 1],
                                             sm[:, H:H + 1])
                        xlT_ps = ptr.tile([1, BL], FP, name=f"xlT_{t}",
                                          tag="tr")
                        nc.tensor.transpose(xlT_ps[:, :], xlab[:, :],
                                            idf[:, :])
                        xlabT = psm.tile([1, BL], BF, name=f"xls_{t}",
                                         tag="s2xls")
                        nc.scalar.copy(xlabT[:, :], xlT_ps[:, :])
                        hT, cT = lstm_finish("s2", t, gps, xsT, cT,
                                             w["wih2T_h"], w["bias2"], BF,
                                             xlabT=xlabT,
                                             wlabT=w["wih2T_l"])
                        hbt = ptr.tile([BL, H], BF, name=f"hbt2_{t}",
                                       tag="tr")
                        nc.tensor.transpose(hbt[:, :], hT[:, :], idb[:, :])
                        nc.scalar.mul(fin_sb[:, t * H:(t + 1) * H],
                                      hbt[:, :], 0.5)

                # =================== DECODER ===================
                with tc.tile_pool(name="pdec", bufs=1) as pdec:
                    ud = pdec.tile([BL, T * H], BF, name="ud", tag="ud")
                    for t in range(T):
                        ftp = ptr.tile([H, BL], BF, name=f"ftp_{t}",
                                       tag="tr")
                        nc.tensor.transpose(ftp[:, :],
                                            fin_sb[:, t * H:(t + 1) * H],
                                            idb[:, :])
                        fts = psm.tile([H, BL], BF, name=f"fts_{t}",
                                       tag="smT_sb")
                        nc.scalar.copy(fts[:, :], ftp[:, :])
                        up = ppsum.tile([BL, H], FP, name=f"udp_{t}",
                                        tag="mm_small")
                        nc.tensor.matmul(up[:, :], fts[:, :], w["udT"][:, :],
                                         start=True, stop=True)
                        nc.vector.scalar_tensor_tensor(
                            ud[:, t * H:(t + 1) * H], up[:, :], 1.0,
                            w["udb_rep"][:, :], op0=OP.mult, op1=OP.add)

                    hT, cT = new_state("sd", FP)
                    hTb = psm.tile([H, BL], BF, name="sdb0", tag="sdb16")
                    nc.scalar.copy(hTb[:, :], hT[:, :])
                    for t in range(ND):
                        wdps = ppsum.tile([BL, H], FP, name=f"wd_{t}",
                                          tag="mm_small")
                        nc.tensor.matmul(wdps[:, :], hTb[:, :],
                                         w["wdT_h"][:, :], start=True,
                                         stop=False)
                        nc.tensor.matmul(wdps[:, :], cT[:, :],
                                         w["wdT_c"][:, :], start=False,
                                         stop=True)
                        gps = lstm_begin("sd", t, hTb, w["whhdT"])
                        wd_sb = psm.tile([BL, H], BF, name=f"wds_{t}",
                                         tag="we_sb")
                        nc.scalar.copy(wd_sb[:, :], wdps[:, :])
                        score = psm.tile([BL, T], FP, name=f"scd_{t}",
                                         tag="sc")
                        eargs, etans = [], []
                        for ci, (t0, tn) in enumerate(TD_CH):
                            earg = pdec.tile([BL, tn * H], BF,
                                             name=f"ead_{t}_{ci}",
                                             tag=f"bigdA{ci}", bufs=2)
                            nc.vector.tensor_add(
                                earg[:, :].rearrange("p (s h) -> p s h",
                                                     h=H),
                                ud[:, t0 * H:(t0 + tn) * H].rearrange(
                                    "p (s h) -> p s h", h=H),
                                wd_sb[:, :].unsqueeze(1).broadcast_to(
                                    [BL, tn, H]))
                            eargs.append(earg)
                        for ci, (t0, tn) in enumerate(TD_CH):
                            etan = pdec.tile([BL, tn * H], BF,
                                             name=f"etd_{t}_{ci}",
                                             tag=f"bigdB{ci}", bufs=2)
                            nc.scalar.activation(etan[:, :],
                                                 eargs[ci][:, :], AF.Tanh)
                            etans.append(etan)
                        for ci, (t0, tn) in enumerate(TD_CH):
                            t2 = pdec.tile([BL, tn * H], BF,
                                           name=f"t2d_{t}_{ci}",
                                           tag=f"bigdA{ci}", bufs=2)
                            nc.vector.tensor_mul(
                                t2[:, :].rearrange("p (s h) -> p s h", h=H),
                                etans[ci][:, :].rearrange(
                                    "p (s h) -> p s h", h=H),
                                w["vd_rep"][:, :].unsqueeze(1).broadcast_to(
                                    [BL, tn, H]))
                            tree_h(pdec, "sd", t, ci, t2, tn,
                                   score[:, t0:t0 + tn])
                        sm = softmax("sd", t, score, T)
                        din = psm.tile([BL, H], FP, name=f"din_{t}",
                                       tag="sddin")
                        for ci, (h0, hn) in enumerate(DIN_CH):
                            dmul = pdec.tile([BL, T * hn], BF,
                                             name=f"dm_{t}_{ci}",
                                             tag="bigdA", bufs=2)
                            nc.vector.tensor_mul(
                                dmul[:, :].rearrange("p (s h) -> p s h",
                                                     h=hn),
                                fin_sb[:, :].rearrange(
                                    "p (s h) -> p s h", h=H)[:, :,
                                                             h0:h0 + hn],
                                sm[:, :].unsqueeze(2).broadcast_to(
                                    [BL, T, hn]))
                            tree_t(pdec, "sd", t, ci, dmul, hn,
                                   din[:, h0:h0 + hn])
                        dinT_ps = ptr.tile([H, BL], FP, name=f"dTp_{t}",
                                           tag="tr")
                        nc.tensor.transpose(dinT_ps[:, :], din[:, :],
                                            idf[:, :])
                        dinT = psm.tile([H, BL], BF, name=f"dT_{t}",
                                        tag="sddinT")
                        nc.scalar.copy(dinT[:, :], dinT_ps[:, :])
                        hT, cT = lstm_finish("sd", t, gps, dinT, cT,
                                             w["wihdT"], w["biasd"], FP)
                        hTb = psm.tile([H, BL], BF, name=f"sdb{t + 1}",
                                       tag="sdb16")
                        nc.scalar.copy(hTb[:, :], hT[:, :])
                        op = ppsum.tile([BL, 1], FP, name=f"op_{t}",
                                        tag="mm_small")
                        nc.tensor.matmul(op[:, :], hT[:, :],
                                         w["regWT"][:, :], start=True,
                                         stop=True)
                        nc.scalar.copy(out_sb[:, t:t + 1], op[:, :])

                    outf = pdec.tile([BL, TD], FP, name="outf", tag="outf")
                    nc.vector.tensor_scalar_add(outf[:, :], out_sb[:, 6:ND],
                                                w["regb_rep"][:, :])
                    nc.gpsimd.dma_start(out=out_d[:, :], in_=outf[:, :])

    if split_waits:
        _split_matmul_waits(nc)
    return nc


# ---------------- host-side prep ----------------
def _prep_weights(i):
    f32 = lambda a: np.ascontiguousarray(a, np.float32)
    try:
        import ml_dtypes
        bf16 = lambda a: np.ascontiguousarray(
            np.asarray(a, np.float32).astype(ml_dtypes.bfloat16))
    except ImportError:
        import jax.numpy as jnp
        bf16 = lambda a: np.ascontiguousarray(
            np.asarray(jnp.asarray(a, jnp.bfloat16)))
    # gate order [i, f, o, g]: i/f/o share the tanh(0.5*(.)) activation
    gperm = np.concatenate([np.arange(0, H), np.arange(H, 2 * H),
                            np.arange(3 * H, 4 * H), np.arange(2 * H, 3 * H)])
    w = {}
    w["we1T_h"] = bf16(0.5 * i["We1_W"][:, :H].T)
    w["we1T_c"] = f32(0.5 * i["We1_W"][:, H:].T)
    w["ue1T"] = bf16(i["Ue1_W"].T)
    w["ue1b_rep"] = f32(np.tile(i["Ue1_b"][None, :], (BL, 1)))
    w["ve1_rep"] = bf16(np.tile(i["Ve1_W"][0][None, :], (BL, 1)))
    w["wih1T"] = bf16(i["e1_Wih"].T[:, gperm])
    w["whh1T"] = bf16(0.5 * i["e1_Whh"].T[:, gperm])
    b1 = i["e1_bih"] + i["e1_bhh"]
    _hi = np.asarray(bf16(b1[gperm]), np.float32)
    w["bias1"] = np.concatenate([bf16(b1[gperm])[None, :],
                               bf16(b1[gperm] - _hi)[None, :]], 0)
    w["we2T_h"] = bf16(0.5 * i["We2_W"][:, :H].T)
    w["we2T_c"] = f32(0.5 * i["We2_W"][:, H:].T)
    w["ue2T"] = bf16(i["Ue2_W"].T)
    w["ue2b_rep"] = f32(np.tile(i["Ue2_b"][None, :], (BL, 1)))
    w["ve2_rep"] = bf16(np.tile(i["Ve2_W"][0][None, :], (BL, 1)))
    w["wih2T_h"] = bf16(i["e2_Wih"][:, :H].T[:, gperm])
    w["wih2T_l"] = bf16(i["e2_Wih"][:, H:].T[:, gperm])
    w["whh2T"] = bf16(0.5 * i["e2_Whh"].T[:, gperm])
    b2 = i["e2_bih"] + i["e2_bhh"]
    _hi = np.asarray(bf16(b2[gperm]), np.float32)
    w["bias2"] = np.concatenate([bf16(b2[gperm])[None, :],
                               bf16(b2[gperm] - _hi)[None, :]], 0)
    w["udT"] = bf16(i["Ud_W"].T)
    w["udb_rep"] = f32(np.tile(i["Ud_b"][None, :], (BL, 1)))
    w["wdT_h"] = bf16(0.5 * i["Wd_W"][:, :H].T)
    w["wdT_c"] = f32(0.5 * i["Wd_W"][:, H:].T)
    w["vd_rep"] = bf16(np.tile(i["Vd_W"][0][None, :], (BL, 1)))
    w["wihdT"] = bf16(i["d_Wih"].T[:, gperm])
    w["whhdT"] = bf16(0.5 * i["d_Whh"].T[:, gperm])
    bd = i["d_bih"] + i["d_bhh"]
    _hi = np.asarray(bf16(bd[gperm]), np.float32)
    w["biasd"] = np.concatenate([bf16(bd[gperm])[None, :],
                               bf16(bd[gperm] - _hi)[None, :]], 0)
    w["regWT"] = f32(0.5 * i["reg_W"].T)
    w["regb_rep"] = f32(np.tile(i["reg_b"][None, :], (BL, 1)))
    w["identf"] = np.eye(128, dtype=np.float32)
    w["ones_row"] = bf16(np.ones((2, BL)))
    w["identb"] = bf16(np.eye(128))
    w["_bf16"] = bf16
    return w


def prep_in_maps(inputs):
    i = {k: np.asarray(v) for k, v in inputs.items()}
    wshared = _prep_weights(i)
    bf16 = wshared.pop("_bf16")
    maps = []
    for c in range(NCORES):
        sl = slice(c * BL, (c + 1) * BL)
        x = np.ascontiguousarray(i["input_p_q"][sl][:, :, 1:], np.float32)
        lab = np.ascontiguousarray(i["label_p"][sl], np.float32)
        m = dict(wshared)
        m["xTT"] = bf16(x.transpose(1, 2, 0).reshape(T, F1 * BL))
        m["xT"] = bf16(x.transpose(2, 1, 0).reshape(F1, T * BL))
        m["lab"] = lab
        maps.append(m)
    return maps


_CACHE = {}


def kernel(**inputs):
    from concourse.bass_utils import run_bass_kernel_spmd
    if "nc" not in _CACHE:
        _CACHE["nc"] = build_bass()
    nc = _CACHE["nc"]
    in_maps = prep_in_maps(inputs)
    res = run_bass_kernel_spmd(nc, in_maps, list(range(NCORES)))
    out = np.concatenate([res.results[c]["out"] for c in range(NCORES)], 0)
    return out.astype(np.float32)


if __name__ == "__main__":
    nc = build_bass()
    print("built ok")
